# revision 1
# baseline (speedup 1.0000x reference)
"""DeltaNet fused kernel for 8 TRN2 NeuronCores (Bass/Tile).

Math (reference):
    s  = x @ W_slow_w.T + W_slow_b            [B, 3073]
    k  = s[:, :1024]; v = s[:, 1024:2048]; q = s[:, 2048:3072]
    lr = sigmoid(s[:, 3072])
    v_bar = softmax(k) @ W_fast_w.T + W_fast_b          (W_fast_w == 0 -> v_bar = W_fast_b)
    deltaT[h,o] = sum_b sigmoid(k)[b,h] * (lr*(v - v_bar))[b,o] / B
    out = softmax(q) @ (W_fast_w + delta).T + W_fast_b

Sharding: data-parallel over batch (2048 rows/core). deltaT partials are
AllReduced across the 8 cores (two AllReduces, one per batch half, so the
first overlaps the second half's compute).

Device layout trick: all matmuls use natural layouts (no on-chip transposes):
  - host pre-transposes x (per-shard) and W_slow_w to bf16
  - q is computed TRANSPOSED (qT[h,b]) so exp(qT) is directly the lhsT of the
    final matmul; softmax denominator comes from a ones-vector matmul column
    and is applied as a per-partition output scale.
"""

import os
import sys

for _p in ("/opt/trn_rl_repo", "/root/.axon_site/_ro/trn_rl_repo"):
    if os.path.isdir(_p) and _p not in sys.path:
        sys.path.append(_p)

import numpy as np
import ml_dtypes

BF16 = ml_dtypes.bfloat16

N_CORES = 8
B_FULL = 16384
DIM = 1024          # dim_in == dim_out == dim_hidden
SLOW_OUT = 3 * DIM + 1
P = 128
NT = DIM // P       # 8 tiles along any 1024 dim


def _build_program(b_core: int, n_cores: int = N_CORES):
    """Build the SPMD Bass program (same program on every core)."""
    import concourse.bass as bass
    import concourse.mybir as mybir
    import concourse.tile as tile
    from concourse import bacc

    f32 = mybir.dt.float32
    bf16 = mybir.dt.bfloat16
    AF = mybir.ActivationFunctionType
    ALU = mybir.AluOpType

    assert b_core % (2 * P) == 0
    nbt = b_core // P            # b-tiles per core
    nbt_h = nbt // 2             # b-tiles per half
    bh = b_core // 2             # rows per half
    n_bchunk_h = bh // 512 if bh >= 512 else 0   # 512-wide chunks per half
    assert bh % 512 == 0, "half batch must be a multiple of 512"

    nc = bacc.Bacc(
        "TRN2",
        target_bir_lowering=False,
        debug=False,
        num_devices=n_cores,
    )

    # ---- kernel I/O ----
    xT_h = nc.dram_tensor("xT", [DIM, b_core], bf16, kind="ExternalInput")
    wT_h = nc.dram_tensor("wT", [DIM, SLOW_OUT], bf16, kind="ExternalInput")
    bk_h = nc.dram_tensor("bk", [DIM], f32, kind="ExternalInput")     # W_slow_b[:1024]
    bvc_h = nc.dram_tensor("bvc", [DIM], f32, kind="ExternalInput")   # W_slow_b[1024:2048] - W_fast_b
    bq_h = nc.dram_tensor("bq", [DIM], f32, kind="ExternalInput")     # W_slow_b[2048:3072]
    blr_h = nc.dram_tensor("blr", [1], f32, kind="ExternalInput")     # W_slow_b[3072]
    wfb_h = nc.dram_tensor("wfb", [DIM], f32, kind="ExternalInput")   # W_fast_b
    out_h = nc.dram_tensor("out", [b_core, DIM], f32, kind="ExternalOutput")

    inv_b = 1.0 / float(b_core * n_cores)

    with tile.TileContext(nc) as tc:
        with (
            tc.tile_pool(name="persist", bufs=1) as persist,
            tc.tile_pool(name="psum", bufs=8, space="PSUM") as psum,
            tc.tile_pool(name="tmp", bufs=4) as tmp,
            tc.tile_pool(name="small", bufs=6) as small,
            tc.tile_pool(name="ost", bufs=2) as ost,
            tc.tile_pool(name="arl", bufs=4) as arl,
            tc.tile_pool(name="dram", bufs=1, space="DRAM") as dram,
        ):
            # ---- persistent SBUF tensors ----
            wt = persist.tile([P, NT, SLOW_OUT], bf16, name="wt")
            xt = persist.tile([P, NT, bh], bf16, name="xt")          # one half at a time
            et = persist.tile([P, NT, b_core], bf16, name="et")      # exp(qT), full batch
            sigk = persist.tile([P, nbt_h, DIM], bf16, name="sigk")  # one half
            u = persist.tile([P, nbt_h, DIM], bf16, name="u")        # one half
            wn = persist.tile([P, NT, DIM], bf16, name="wn")         # W_new.T
            dstage = persist.tile([P, NT, DIM], bf16, name="dstage")
            bk_b = persist.tile([P, DIM], f32, name="bk_b")
            bvc_b = persist.tile([P, DIM], f32, name="bvc_b")
            wfb_b = persist.tile([P, DIM], f32, name="wfb_b")
            bq_c = persist.tile([P, NT], f32, name="bq_c")
            blr_c = persist.tile([P, 1], f32, name="blr_c")
            ones = persist.tile([P, 1], bf16, name="ones")

            # ---- DRAM bounce buffers for the two AllReduces ----
            ar_in = [
                dram.tile([DIM, DIM], bf16, name=f"ar_in{h}") for h in range(2)
            ]
            ar_out = [
                dram.tile([DIM, DIM], bf16, name=f"ar_out{h}", addr_space="Shared")
                for h in range(2)
            ]

            # ---- constants / weights ----
            nc.vector.memset(ones[:], 1.0)
            for bias_dst, bias_src in ((bk_b, bk_h), (bvc_b, bvc_h), (wfb_b, wfb_h)):
                nc.gpsimd.dma_start(
                    out=bias_dst[:],
                    in_=bass.AP(tensor=bias_src, offset=0, ap=[[0, P], [1, DIM]]),
                )
            # bq_c[p, i] = bq[i*128 + p]
            nc.gpsimd.dma_start(
                out=bq_c[:],
                in_=bass.AP(tensor=bq_h, offset=0, ap=[[1, P], [P, NT]]),
            )
            nc.gpsimd.dma_start(
                out=blr_c[:],
                in_=bass.AP(tensor=blr_h, offset=0, ap=[[0, P], [1, 1]]),
            )
            for i in range(NT):
                nc.sync.dma_start(out=wt[:, i, :], in_=wT_h[i * P:(i + 1) * P, :])

            for half in range(2):
                # ---- load this half of xT ----
                for i in range(NT):
                    nc.sync.dma_start(
                        out=xt[:, i, :],
                        in_=xT_h[i * P:(i + 1) * P, half * bh:(half + 1) * bh],
                    )

                # ---- s-phase: k / v / lr per b-tile ----
                for t in range(nbt_h):
                    ps = [
                        psum.tile([P, 512], f32, tag="ps", name=f"ps{half}_{t}_{c}")
                        for c in range(4)
                    ]
                    plr = psum.tile([P, 1], f32, tag="ps", name=f"plr{half}_{t}")
                    for i in range(NT):
                        lhs = xt[:, i, t * P:(t + 1) * P]
                        st = dict(start=(i == 0), stop=(i == NT - 1))
                        for c in range(4):
                            nc.tensor.matmul(
                                ps[c][:], lhs, wt[:, i, c * 512:(c + 1) * 512], **st
                            )
                        nc.tensor.matmul(
                            plr[:], lhs, wt[:, i, 3 * DIM:3 * DIM + 1], **st
                        )
                    # lr = sigmoid(plr + blr) / B
                    lr_s = small.tile([P, 1], f32, tag="lr", name=f"lr{half}_{t}")
                    nc.scalar.activation(lr_s[:], plr[:], AF.Sigmoid, bias=blr_c[:, 0:1])
                    nc.vector.tensor_scalar_mul(lr_s[:], lr_s[:], inv_b)
                    for c in range(2):
                        # sigk = sigmoid(k + bk)
                        ktmp = tmp.tile([P, 512], f32, tag="kv", name=f"kt{half}_{t}_{c}")
                        nc.vector.tensor_add(ktmp[:], ps[c][:], bk_b[:, c * 512:(c + 1) * 512])
                        nc.scalar.activation(
                            sigk[:, t, c * 512:(c + 1) * 512], ktmp[:], AF.Sigmoid
                        )
                        # u = lr/B * (v + (bv - wfb))
                        vtmp = tmp.tile([P, 512], f32, tag="kv", name=f"vt{half}_{t}_{c}")
                        nc.vector.tensor_add(
                            vtmp[:], ps[2 + c][:], bvc_b[:, c * 512:(c + 1) * 512]
                        )
                        nc.scalar.activation(
                            u[:, t, c * 512:(c + 1) * 512], vtmp[:], AF.Copy,
                            scale=lr_s[:],
                        )

                # ---- delta-phase: deltaT_half[h, o] = sum_b sigk * u ----
                for hh in range(NT):
                    pd = [
                        psum.tile([P, 512], f32, tag="ps", name=f"pd{half}_{hh}_{oc}")
                        for oc in range(2)
                    ]
                    for t in range(nbt_h):
                        st = dict(start=(t == 0), stop=(t == nbt_h - 1))
                        lhs = sigk[:, t, hh * P:(hh + 1) * P]
                        for oc in range(2):
                            nc.tensor.matmul(
                                pd[oc][:], lhs, u[:, t, oc * 512:(oc + 1) * 512], **st
                            )
                    for oc in range(2):
                        nc.vector.tensor_copy(
                            dstage[:, hh, oc * 512:(oc + 1) * 512], pd[oc][:]
                        )
                    nc.sync.dma_start(
                        out=ar_in[half][hh * P:(hh + 1) * P, :], in_=dstage[:, hh, :]
                    )
                nc.gpsimd.collective_compute(
                    "AllReduce",
                    mybir.AluOpType.add,
                    replica_groups=[list(range(n_cores))],
                    ins=[ar_in[half][:, :]],
                    outs=[ar_out[half][:, :]],
                )

                # ---- q-phase: et = exp(qT + bq) (transposed layout) ----
                for hh in range(NT):
                    for bc in range(n_bchunk_h):
                        pq = psum.tile(
                            [P, 512], f32, tag="ps", name=f"pq{half}_{hh}_{bc}"
                        )
                        for i in range(NT):
                            nc.tensor.matmul(
                                pq[:],
                                wt[:, i, 2 * DIM + hh * P:2 * DIM + (hh + 1) * P],
                                xt[:, i, bc * 512:(bc + 1) * 512],
                                start=(i == 0),
                                stop=(i == NT - 1),
                            )
                        nc.scalar.activation(
                            et[:, hh, half * bh + bc * 512:half * bh + (bc + 1) * 512],
                            pq[:],
                            AF.Exp,
                            bias=bq_c[:, hh:hh + 1],
                        )

            # ---- W_newT = ar_out[0] + ar_out[1]  (bf16) ----
            for hh in range(NT):
                a0 = arl.tile([P, DIM], bf16, tag="ar", name=f"a0_{hh}")
                a1 = arl.tile([P, DIM], bf16, tag="ar", name=f"a1_{hh}")
                nc.sync.dma_start(out=a0[:], in_=ar_out[0][hh * P:(hh + 1) * P, :])
                nc.sync.dma_start(out=a1[:], in_=ar_out[1][hh * P:(hh + 1) * P, :])
                nc.vector.tensor_add(wn[:, hh, :], a0[:], a1[:])

            # ---- final: out = (et.T @ wn) / rowsum + wfb ----
            for t in range(nbt):
                po = [
                    psum.tile([P, 512], f32, tag="ps", name=f"po{t}_{oc}")
                    for oc in range(2)
                ]
                prs = psum.tile([P, 1], f32, tag="ps", name=f"prs{t}")
                for hh in range(NT):
                    lhs = et[:, hh, t * P:(t + 1) * P]
                    st = dict(start=(hh == 0), stop=(hh == NT - 1))
                    for oc in range(2):
                        nc.tensor.matmul(
                            po[oc][:], lhs, wn[:, hh, oc * 512:(oc + 1) * 512], **st
                        )
                    nc.tensor.matmul(prs[:], lhs, ones[:], **st)
                recip = small.tile([P, 1], f32, tag="rc", name=f"rc{t}")
                nc.vector.reciprocal(recip[:], prs[:])
                o_st = ost.tile([P, DIM], f32, tag="os", name=f"os{t}")
                for oc in range(2):
                    nc.vector.scalar_tensor_tensor(
                        o_st[:, oc * 512:(oc + 1) * 512],
                        po[oc][:],
                        recip[:],
                        wfb_b[:, oc * 512:(oc + 1) * 512],
                        op0=ALU.mult,
                        op1=ALU.add,
                    )
                nc.sync.dma_start(out=out_h[t * P:(t + 1) * P, :], in_=o_st[:])

    nc.compile()
    return nc


def _host_prep(x, W_slow_w, W_slow_b, W_fast_b, b_core, n_cores):
    """Shard + pre-transpose + cast inputs; returns per-core input maps."""
    wT = np.ascontiguousarray(W_slow_w.T).astype(BF16)
    bk = np.ascontiguousarray(W_slow_b[:DIM]).astype(np.float32)
    bvc = (W_slow_b[DIM:2 * DIM] - W_fast_b).astype(np.float32)
    bq = np.ascontiguousarray(W_slow_b[2 * DIM:3 * DIM]).astype(np.float32)
    blr = np.ascontiguousarray(W_slow_b[3 * DIM:3 * DIM + 1]).astype(np.float32)
    wfb = np.ascontiguousarray(W_fast_b).astype(np.float32)
    in_maps = []
    for c in range(n_cores):
        xs = x[c * b_core:(c + 1) * b_core, :]
        xT = np.ascontiguousarray(xs.T).astype(BF16)
        in_maps.append(
            {"xT": xT, "wT": wT, "bk": bk, "bvc": bvc, "bq": bq, "blr": blr,
             "wfb": wfb}
        )
    return in_maps


_PROGRAM_CACHE = {}


def _get_program(b_core, n_cores=N_CORES):
    key = (b_core, n_cores)
    if key not in _PROGRAM_CACHE:
        _PROGRAM_CACHE[key] = _build_program(b_core, n_cores)
    return _PROGRAM_CACHE[key]


def _run_device(x, W_slow_w, W_slow_b, W_fast_b, trace=False):
    from concourse.bass_utils import run_bass_kernel_spmd

    b_core = x.shape[0] // N_CORES
    nc = _get_program(b_core)
    in_maps = _host_prep(x, W_slow_w, W_slow_b, W_fast_b, b_core, N_CORES)
    res = run_bass_kernel_spmd(nc, in_maps, list(range(N_CORES)), trace=trace)
    out = np.concatenate([res.results[c]["out"] for c in range(N_CORES)], axis=0)
    return out.astype(np.float32), res


def _reference_numpy(x, W_slow_w, W_slow_b, W_fast_w, W_fast_b):
    """Exact fallback (only used if W_fast_w != 0, which the spec never produces)."""
    x = x.astype(np.float64)
    s = x @ W_slow_w.astype(np.float64).T + W_slow_b.astype(np.float64)
    k = s[:, :DIM]
    v = s[:, DIM:2 * DIM]
    q = s[:, 2 * DIM:3 * DIM]
    lr = 1.0 / (1.0 + np.exp(-s[:, -1:]))
    ek = np.exp(k - k.max(axis=1, keepdims=True))
    ak = ek / ek.sum(axis=1, keepdims=True)
    v_bar = ak @ W_fast_w.astype(np.float64).T + W_fast_b.astype(np.float64)
    sigk = 1.0 / (1.0 + np.exp(-k))
    delta = (lr * (v - v_bar)).T @ sigk / x.shape[0]
    w_new = W_fast_w.astype(np.float64) + delta
    eq = np.exp(q - q.max(axis=1, keepdims=True))
    aq = eq / eq.sum(axis=1, keepdims=True)
    return (aq @ w_new.T + W_fast_b.astype(np.float64)).astype(np.float32)


def kernel(x, W_slow_w, W_slow_b, W_fast_w, W_fast_b):
    x = np.asarray(x)
    W_slow_w = np.asarray(W_slow_w)
    W_slow_b = np.asarray(W_slow_b)
    W_fast_w = np.asarray(W_fast_w)
    W_fast_b = np.asarray(W_fast_b)
    if np.any(W_fast_w):
        # Spec guarantees W_fast_w == 0; exact fallback for generality.
        return _reference_numpy(x, W_slow_w, W_slow_b, W_fast_w, W_fast_b)
    out, _ = _run_device(x, W_slow_w, W_slow_b, W_fast_b, trace=False)
    return out
